# revision 11
# baseline (speedup 1.0000x reference)
import sys

sys.path.insert(0, "/opt/trn_rl_repo")

from contextlib import ExitStack

import ml_dtypes
import numpy as np

from concourse import bass, mybir, tile
from concourse.bass_utils import run_bass_kernel_spmd
from concourse.vector_clock import ScopedClock


def _patched_drain_and_barrier(self, tick_clock, wait_clock):
    # Workaround: this compiler rejects a drain carrying >1 sem wait
    # ([NCC_INLA001]); split extra waits onto single-wait nops.
    drain_inst = self.nc.sync.drain()
    wait_clock.add_sem_waits(
        drain_inst.ins, ScopedClock({None: tick_clock.global_clock})
    )
    si = drain_inst.ins.sync_info
    waits = list(si.on_wait) if si and si.on_wait else []
    if len(waits) > 1:
        drain_inst.ins.sync_info = mybir.SyncInfo(
            on_wait=[waits[0]], on_update=list(si.on_update or [])
        )
        for w in waits[1:]:
            nop = self.nc.sync.nop(nofuse=True)
            nop.ins.sync_info = mybir.SyncInfo(on_wait=[w], on_update=[])
    self.nc.all_engine_barrier()
    popped = self.nc._tile_sem_poison_stack.pop()
    assert popped is self._sem_poison
    self.nc.clear_and_free_semaphores(list(self.sems.allocated().values()))
    self.nc.all_engine_barrier()


tile.TileContext._drain_and_barrier = _patched_drain_and_barrier


def _split_excess_waits(nc, limit=1):
    # Workaround: this compiler allows only one sem wait on several
    # instruction encodings; move extra waits onto same-engine nops.
    eng_map = {
        mybir.EngineType.PE: nc.tensor,
        mybir.EngineType.Activation: nc.scalar,
        mybir.EngineType.DVE: nc.vector,
        mybir.EngineType.Pool: nc.gpsimd,
        mybir.EngineType.SP: nc.sync,
    }
    for blk in nc.cur_f.blocks:
        orig = list(blk.instructions)
        out = []
        for ins in orig:
            si = ins.sync_info
            waits = list(si.on_wait) if si and si.on_wait else []
            eng = eng_map.get(ins.engine)
            if len(waits) > limit and eng is not None:
                extra, keep = waits[:-limit], waits[-limit:]
                for w in extra:
                    nop = eng.nop(nofuse=True).ins
                    tail = nc.cur_f.blocks[-1].instructions
                    assert tail[-1] is nop
                    tail.pop()
                    nop.sync_info = mybir.SyncInfo(on_wait=[w], on_update=[])
                    out.append(nop)
                ins.sync_info = mybir.SyncInfo(
                    on_wait=keep, on_update=list(si.on_update or [])
                )
            out.append(ins)
        blk.instructions[:] = out


bf16 = ml_dtypes.bfloat16
f8e4 = ml_dtypes.float8_e4m3
BF = bass.mybir.dt.bfloat16
F8 = bass.mybir.dt.float8e4
F32 = bass.mybir.dt.float32
AF = mybir.ActivationFunctionType
ALU = mybir.AluOpType
DR = mybir.MatmulPerfMode.DoubleRow

B, S, E, H, D = 2, 2048, 2048, 16, 128
BS = B * S
NCORES = 8
HPC = H // NCORES  # heads per core
DC = HPC * D  # per-core head-dim width (256)
SCALE = 1.0 / float(np.sqrt(D))
AQ = SCALE * 128.0  # scale folded into Wq/bq so fp8 range is healthy
AK = 64.0  # scale folded into Wk/bk
EXP_SCALE = 1.0 / (128.0 * 64.0)

TRACE = False
LAST_RESULTS = None
_NC_CACHE = None


def _build():
    nc = bass.Bass()
    x8d = nc.declare_dram_parameter("x8", (E, BS), F8, isOutput=False)
    xbd = nc.declare_dram_parameter("xb", (E, BS), BF, isOutput=False)
    wq8d = nc.declare_dram_parameter("wq8", (E, DC), F8, isOutput=False)
    wk8d = nc.declare_dram_parameter("wk8", (E, DC), F8, isOutput=False)
    wvd = nc.declare_dram_parameter("wvT", (E, DC), BF, isOutput=False)
    wod = nc.declare_dram_parameter("woT", (DC, E), BF, isOutput=False)
    bqd = nc.declare_dram_parameter("bq", (DC, 1), F32, isOutput=False)
    bkd = nc.declare_dram_parameter("bk", (DC, 1), F32, isOutput=False)
    negd = nc.declare_dram_parameter("negm", (128, 128), BF, isOutput=False)
    idnd = nc.declare_dram_parameter("ident", (128, 128), BF, isOutput=False)
    onkd = nc.declare_dram_parameter("onesk", (128, 128), BF, isOutput=False)
    yd = nc.declare_dram_parameter("y", (BS, E), BF, isOutput=True)

    with ExitStack() as ctx:
        tc = ctx.enter_context(tile.TileContext(nc))
        wp = ctx.enter_context(tc.tile_pool(name="wp", bufs=1))
        xp = ctx.enter_context(tc.tile_pool(name="xp", bufs=1))
        bp = ctx.enter_context(tc.tile_pool(name="bp", bufs=1))
        pp = ctx.enter_context(tc.tile_pool(name="pp", bufs=8))
        ap_ = ctx.enter_context(tc.tile_pool(name="ap", bufs=2))
        dp = ctx.enter_context(tc.tile_pool(name="dp", bufs=2))
        yp = ctx.enter_context(tc.tile_pool(name="yp", bufs=8))
        ps = ctx.enter_context(tc.tile_pool(name="ps", bufs=1, space="PSUM"))

        # round-robin PSUM tag allocator for projection/output phases
        _prr = [("sc", 3), ("cx", 2), ("dn", 2)]
        _pctr = [0]

        def projps():
            tag, bufs = _prr[_pctr[0] % 3]
            _pctr[0] += 1
            return ps.tile([128, 512], F32, tag=tag, bufs=bufs, name=f"pj_{tag}")

        wq8_sb = wp.tile([128, 16, DC], F8)
        wk8_sb = wp.tile([128, 16, DC], F8)
        wv_sb = wp.tile([128, 16, DC], BF)
        wo_sb = wp.tile([128, HPC, E], BF)
        bq_sb = wp.tile([128, HPC, 1], F32)
        bk_sb = wp.tile([128, HPC, 1], F32)
        negm_sb = wp.tile([128, 128], BF)
        idnt_sb = wp.tile([128, 128], BF)
        onk_sb = wp.tile([128, 128], BF)

        x8_sb = xp.tile([128, 16, S], F8)
        xb_sb = xp.tile([128, 16, S], BF)
        qT_sb = bp.tile([128, HPC, S], BF)
        kT_sb = bp.tile([128, HPC, S], BF)
        v_sb = bp.tile([128, 16, DC], BF)
        ctxN_sb = bp.tile([128, HPC, S], BF)

        for b in range(B):
            s0 = b * S
            # --- input loads (b=0: weights interleaved for fast start) ---
            # spread DMA dispatch across engine queues: SP paces x8 (the
            # critical path), Pool paces weights, both feed the first chains
            for t in range(16):
                nc.sync.dma_start(x8_sb[:, t, :], x8d[t * 128 : (t + 1) * 128, s0 : s0 + S])
                if b == 0:
                    nc.gpsimd.dma_start(wq8_sb[:, t, :], wq8d[t * 128 : (t + 1) * 128, :])
                    nc.gpsimd.dma_start(wk8_sb[:, t, :], wk8d[t * 128 : (t + 1) * 128, :])
            if b == 0:
                for hh in range(HPC):
                    nc.gpsimd.dma_start(bq_sb[:, hh, :], bqd[hh * 128 : (hh + 1) * 128, :])
                    nc.gpsimd.dma_start(bk_sb[:, hh, :], bkd[hh * 128 : (hh + 1) * 128, :])
                nc.gpsimd.dma_start(onk_sb[:], onkd[:])
                nc.gpsimd.dma_start(negm_sb[:], negd[:])
                nc.gpsimd.dma_start(idnt_sb[:], idnd[:])
            for t in range(16):
                (nc.sync if t % 2 == 0 else nc.gpsimd).dma_start(
                    xb_sb[:, t, :], xbd[t * 128 : (t + 1) * 128, s0 : s0 + S]
                )
                if b == 0:
                    nc.sync.dma_start(wv_sb[:, t, :], wvd[t * 128 : (t + 1) * 128, :])
            if b == 0:
                for hh in range(HPC):
                    nc.gpsimd.dma_start(wo_sb[:, hh, :], wod[hh * 128 : (hh + 1) * 128, :])

            # --- q/k projections: fp8 DoubleRow, contract E in 8 pairs ---
            for m in range(HPC):
                for j in range(4):
                    js = slice(j * 512, (j + 1) * 512)
                    q_ps = projps()
                    for tp in range(8):
                        nc.tensor.matmul(
                            q_ps[:],
                            wq8_sb[:, 2 * tp : 2 * tp + 2, m * 128 : (m + 1) * 128],
                            x8_sb[:, 2 * tp : 2 * tp + 2, js],
                            start=(tp == 0),
                            stop=(tp == 7),
                            perf_mode=DR,
                        )
                    nc.vector.tensor_scalar_add(qT_sb[:, m, js], q_ps[:], bq_sb[:, m, :])
                    k_ps = projps()
                    for tp in range(8):
                        nc.tensor.matmul(
                            k_ps[:],
                            wk8_sb[:, 2 * tp : 2 * tp + 2, m * 128 : (m + 1) * 128],
                            x8_sb[:, 2 * tp : 2 * tp + 2, js],
                            start=(tp == 0),
                            stop=(tp == 7),
                            perf_mode=DR,
                        )
                    nc.scalar.activation(
                        kT_sb[:, m, js], k_ps[:], AF.Identity, bias=bk_sb[:, m, :]
                    )

            # --- v projection: bf16, x chunks stationary ---
            for si in range(16):
                v_ps = projps()
                for t in range(16):
                    nc.tensor.matmul(
                        v_ps[:, :DC],
                        xb_sb[:, t, si * 128 : (si + 1) * 128],
                        wv_sb[:, t, :],
                        start=(t == 0),
                        stop=(t == 15),
                    )
                nc.vector.tensor_copy(v_sb[:, si, :], v_ps[:, :DC])

            # --- output projection tile emitter (contract 2 head chunks) ---
            def emit_outproj(qc, interleaved):
                if interleaved:
                    y_ps = ps.tile([128, 512], F32, tag="yo", bufs=1, name="pj_yo")
                else:
                    y_ps = projps()
                eb = qc % 4
                qq = qc // 4 * 4 + 0  # placeholder, unused
                return y_ps, eb

            def outproj_group(qcs, interleaved):
                for qc in qcs:
                    y_t = yp.tile([128, 4, 512], BF, bufs=4, name="y_t")
                    for eb in range(4):
                        if interleaved:
                            y_ps = ps.tile([128, 512], F32, tag="yo", bufs=1, name="pj_yo")
                        else:
                            y_ps = projps()
                        nc.tensor.matmul(
                            y_ps[:],
                            ctxN_sb[:, 0, qc * 128 : (qc + 1) * 128],
                            wo_sb[:, 0, eb * 512 : (eb + 1) * 512],
                            start=True,
                            stop=False,
                        )
                        nc.tensor.matmul(
                            y_ps[:],
                            ctxN_sb[:, 1, qc * 128 : (qc + 1) * 128],
                            wo_sb[:, 1, eb * 512 : (eb + 1) * 512],
                            start=False,
                            stop=True,
                        )
                        if interleaved or (qc * 4 + eb) % 2 == 0:
                            nc.vector.tensor_copy(y_t[:, eb, :], y_ps[:])
                        else:
                            nc.scalar.copy(y_t[:, eb, :], y_ps[:])
                    nc.gpsimd.dma_start(
                        yd[s0 + qc * 128 : s0 + (qc + 1) * 128, :],
                        y_t[:].rearrange("p a b -> p (a b)"),
                    )

            # --- causal attention, scores kept transposed [k, q] ---
            LAG = 3
            for h in range(HPC):
                hd = slice(h * 128, (h + 1) * 128)
                for qb in range(4):
                    qs = slice(qb * 512, (qb + 1) * 512)
                    kmax = 4 * qb + 4
                    offs = [128 * (kc - 4 * qb) if kc >= 4 * qb else 0 for kc in range(kmax)]
                    pts = [None] * kmax
                    ctx_ps = ps.tile([128, 512], F32, tag="cx", bufs=2)
                    den_ps = ps.tile([128, 512], F32, tag="dn", bufs=2)
                    # interleave den/ctx accumulation (lagging LAG tiles)
                    # between score matmuls so PE never waits on ACT exp
                    for kc in range(kmax + LAG):
                        if kc < kmax:
                            off = offs[kc]
                            diag = kc >= 4 * qb
                            sc_ps = ps.tile([128, 512], F32, tag="sc", bufs=3)
                            nc.tensor.matmul(
                                sc_ps[:, off:],
                                kT_sb[:, h, kc * 128 : (kc + 1) * 128],
                                qT_sb[:, h, qb * 512 + off : (qb + 1) * 512],
                                start=True,
                                stop=not diag,
                            )
                            if diag:
                                # causal mask: only the leading 128 cols of a
                                # diag tile are triangular; add -1e9 there
                                nc.tensor.matmul(
                                    sc_ps[:, off : off + 128],
                                    idnt_sb[:],
                                    negm_sb[:],
                                    start=False,
                                    stop=True,
                                )
                            p_t = pp.tile([128, 512], BF)
                            nc.scalar.activation(
                                p_t[:, off:], sc_ps[:, off:], AF.Exp, scale=EXP_SCALE
                            )
                            pts[kc] = p_t
                        j = kc - LAG
                        if 0 <= j < kmax:
                            joff = offs[j]
                            nc.tensor.matmul(
                                den_ps[:, joff:],
                                onk_sb[:],
                                pts[j][:, joff:],
                                start=(j == 0),
                                stop=(j == kmax - 1),
                            )
                            nc.tensor.matmul(
                                ctx_ps[:, joff:],
                                v_sb[:, j, hd],
                                pts[j][:, joff:],
                                start=(j == 0),
                                stop=(j == kmax - 1),
                            )
                    lnd = dp.tile([128, 512], F32, tag="lnd", bufs=2)
                    nc.scalar.activation(lnd[:], den_ps[:], AF.Ln)
                    recb = dp.tile([128, 512], F32, tag="recb", bufs=2)
                    nc.scalar.activation(recb[:], lnd[:], AF.Exp, scale=-1.0)
                    nc.vector.tensor_tensor(
                        ctxN_sb[:, h, qs], ctx_ps[:], recb[:], ALU.mult
                    )
                    if b == 1 and h == 1:
                        outproj_group(range(4 * qb, 4 * qb + 4), interleaved=True)

            if b == 0:
                outproj_group(range(16), interleaved=False)
    _split_excess_waits(nc)
    return nc


def _negm_np():
    kk = np.arange(128)[:, None]
    qq = np.arange(128)[None, :]
    return np.where(kk > qq, -1.0e9, 0.0).astype(bf16)


def kernel(**inputs):
    global LAST_RESULTS, _NC_CACHE
    x = np.asarray(inputs["x"], np.float32)
    Wq = np.asarray(inputs["Wq"], np.float32)
    bq = np.asarray(inputs["bq"], np.float32)
    Wk = np.asarray(inputs["Wk"], np.float32)
    bk = np.asarray(inputs["bk"], np.float32)
    Wv = np.asarray(inputs["Wv"], np.float32)
    bv = np.asarray(inputs["bv"], np.float32)
    Wo = np.asarray(inputs["Wo"], np.float32)
    bo = np.asarray(inputs["bo"], np.float32)

    xT = np.ascontiguousarray(x.reshape(BS, E).T)
    x8 = xT.astype(f8e4)
    xb = xT.astype(bf16)
    negm = _negm_np()
    ident = np.eye(128, dtype=bf16)
    onesk = np.ones((128, 128), bf16)

    in_maps = []
    for c in range(NCORES):
        dsl = slice(c * DC, (c + 1) * DC)
        in_maps.append(
            {
                "x8": x8,
                "xb": xb,
                "wq8": np.ascontiguousarray(Wq[dsl].T * AQ).astype(f8e4),
                "wk8": np.ascontiguousarray(Wk[dsl].T * AK).astype(f8e4),
                "wvT": np.ascontiguousarray(Wv[dsl].T).astype(bf16),
                "woT": np.ascontiguousarray(Wo[:, dsl].T).astype(bf16),
                "bq": np.ascontiguousarray((bq[dsl] * AQ).reshape(DC, 1)),
                "bk": np.ascontiguousarray((bk[dsl] * AK).reshape(DC, 1)),
                "negm": negm,
                "ident": ident,
                "onesk": onesk,
            }
        )

    if _NC_CACHE is None:
        _NC_CACHE = _build()
    res = run_bass_kernel_spmd(_NC_CACHE, in_maps, core_ids=list(range(NCORES)), trace=TRACE)
    LAST_RESULTS = res

    acc = None
    for r in res.results:
        yc = np.asarray(r["y"], np.float32)
        acc = yc if acc is None else acc + yc
    bo_eff = bo + bv @ Wo.T
    acc += bo_eff[None, :]
    return acc.reshape(B, S, E).astype(np.float32)


# revision 12
# speedup vs baseline: 1.2146x; 1.2146x over previous
import sys

sys.path.insert(0, "/opt/trn_rl_repo")

from contextlib import ExitStack

import ml_dtypes
import numpy as np

from concourse import bass, mybir, tile
from concourse.bass_utils import run_bass_kernel_spmd
from concourse.vector_clock import ScopedClock


def _patched_drain_and_barrier(self, tick_clock, wait_clock):
    # Workaround: this compiler rejects a drain carrying >1 sem wait
    # ([NCC_INLA001]); split extra waits onto single-wait nops.
    drain_inst = self.nc.sync.drain()
    wait_clock.add_sem_waits(
        drain_inst.ins, ScopedClock({None: tick_clock.global_clock})
    )
    si = drain_inst.ins.sync_info
    waits = list(si.on_wait) if si and si.on_wait else []
    if len(waits) > 1:
        drain_inst.ins.sync_info = mybir.SyncInfo(
            on_wait=[waits[0]], on_update=list(si.on_update or [])
        )
        for w in waits[1:]:
            nop = self.nc.sync.nop(nofuse=True)
            nop.ins.sync_info = mybir.SyncInfo(on_wait=[w], on_update=[])
    self.nc.all_engine_barrier()
    popped = self.nc._tile_sem_poison_stack.pop()
    assert popped is self._sem_poison
    self.nc.clear_and_free_semaphores(list(self.sems.allocated().values()))
    self.nc.all_engine_barrier()


tile.TileContext._drain_and_barrier = _patched_drain_and_barrier


def _split_excess_waits(nc, limit=1):
    # Workaround: this compiler allows only one sem wait on several
    # instruction encodings; move extra waits onto same-engine nops.
    eng_map = {
        mybir.EngineType.PE: nc.tensor,
        mybir.EngineType.Activation: nc.scalar,
        mybir.EngineType.DVE: nc.vector,
        mybir.EngineType.Pool: nc.gpsimd,
        mybir.EngineType.SP: nc.sync,
    }
    for blk in nc.cur_f.blocks:
        orig = list(blk.instructions)
        out = []
        for ins in orig:
            si = ins.sync_info
            waits = list(si.on_wait) if si and si.on_wait else []
            eng = eng_map.get(ins.engine)
            if len(waits) > limit and eng is not None:
                extra, keep = waits[:-limit], waits[-limit:]
                for w in extra:
                    nop = eng.nop(nofuse=True).ins
                    tail = nc.cur_f.blocks[-1].instructions
                    assert tail[-1] is nop
                    tail.pop()
                    nop.sync_info = mybir.SyncInfo(on_wait=[w], on_update=[])
                    out.append(nop)
                ins.sync_info = mybir.SyncInfo(
                    on_wait=keep, on_update=list(si.on_update or [])
                )
            out.append(ins)
        blk.instructions[:] = out


bf16 = ml_dtypes.bfloat16
f8e4 = ml_dtypes.float8_e4m3
BF = bass.mybir.dt.bfloat16
F8 = bass.mybir.dt.float8e4
F32 = bass.mybir.dt.float32
AF = mybir.ActivationFunctionType
ALU = mybir.AluOpType
DR = mybir.MatmulPerfMode.DoubleRow

B, S, E, H, D = 2, 2048, 2048, 16, 128
BS = B * S
NCORES = 8
HPC = H // NCORES  # heads per core
DC = HPC * D  # per-core head-dim width (256)
SCALE = 1.0 / float(np.sqrt(D))
AQ = SCALE * 128.0  # scale folded into Wq/bq so fp8 range is healthy
AK = 64.0  # scale folded into Wk/bk
EXP_SCALE = 1.0 / (128.0 * 64.0)

TRACE = False
LAST_RESULTS = None
_NC_CACHE = None


def _build():
    nc = bass.Bass()
    x8d = nc.declare_dram_parameter("x8", (E, BS), F8, isOutput=False)
    xbd = nc.declare_dram_parameter("xb", (E, BS), BF, isOutput=False)
    wq8d = nc.declare_dram_parameter("wq8", (E, DC), F8, isOutput=False)
    wk8d = nc.declare_dram_parameter("wk8", (E, DC), F8, isOutput=False)
    wvd = nc.declare_dram_parameter("wvT", (E, DC), BF, isOutput=False)
    wod = nc.declare_dram_parameter("woT", (DC, E), BF, isOutput=False)
    bqd = nc.declare_dram_parameter("bq", (DC, 1), F32, isOutput=False)
    bkd = nc.declare_dram_parameter("bk", (DC, 1), F32, isOutput=False)
    negd = nc.declare_dram_parameter("negm", (128, 128), BF, isOutput=False)
    idnd = nc.declare_dram_parameter("ident", (128, 128), BF, isOutput=False)
    onkd = nc.declare_dram_parameter("onesk", (128, 128), BF, isOutput=False)
    yd = nc.declare_dram_parameter("y", (BS, E), BF, isOutput=True)

    with ExitStack() as ctx:
        tc = ctx.enter_context(tile.TileContext(nc))
        wp = ctx.enter_context(tc.tile_pool(name="wp", bufs=1))
        xp = ctx.enter_context(tc.tile_pool(name="xp", bufs=1))
        bp = ctx.enter_context(tc.tile_pool(name="bp", bufs=1))
        pp = ctx.enter_context(tc.tile_pool(name="pp", bufs=8))
        ap_ = ctx.enter_context(tc.tile_pool(name="ap", bufs=2))
        dp = ctx.enter_context(tc.tile_pool(name="dp", bufs=2))
        yp = ctx.enter_context(tc.tile_pool(name="yp", bufs=8))
        ps = ctx.enter_context(tc.tile_pool(name="ps", bufs=1, space="PSUM"))

        # round-robin PSUM tag allocator for projection/output phases
        _prr = [("sc", 4), ("cx", 2), ("dn", 2)]
        _pctr = [0]

        def projps():
            tag, bufs = _prr[_pctr[0] % 3]
            _pctr[0] += 1
            return ps.tile([128, 512], F32, tag=tag, bufs=bufs, name=f"pj_{tag}")

        wq8_sb = wp.tile([128, 16, DC], F8)
        wk8_sb = wp.tile([128, 16, DC], F8)
        wv_sb = wp.tile([128, 16, DC], BF)
        wo_sb = wp.tile([128, HPC, E], BF)
        bq_sb = wp.tile([128, HPC, 1], F32)
        bk_sb = wp.tile([128, HPC, 1], F32)
        negm_sb = wp.tile([128, 128], BF)
        idnt_sb = wp.tile([128, 128], BF)
        onk_sb = wp.tile([128, 128], BF)

        x8_sb = xp.tile([128, 16, S], F8)
        xb_sb = xp.tile([128, 16, S], BF)
        qT_sb = bp.tile([128, HPC, S], BF)
        kT_sb = bp.tile([128, HPC, S], BF)
        v_sb = bp.tile([128, 16, DC], BF)
        ctxN_sb = bp.tile([128, HPC, S], BF)

        for b in range(B):
            s0 = b * S
            # --- input loads (b=0: weights interleaved for fast start) ---
            # spread DMA dispatch across engine queues: SP paces x8 (the
            # critical path), Pool paces weights, both feed the first chains
            for t in range(16):
                nc.sync.dma_start(x8_sb[:, t, :], x8d[t * 128 : (t + 1) * 128, s0 : s0 + S])
                if b == 0:
                    nc.gpsimd.dma_start(wq8_sb[:, t, :], wq8d[t * 128 : (t + 1) * 128, :])
                    nc.gpsimd.dma_start(wk8_sb[:, t, :], wk8d[t * 128 : (t + 1) * 128, :])
            if b == 0:
                for hh in range(HPC):
                    nc.gpsimd.dma_start(bq_sb[:, hh, :], bqd[hh * 128 : (hh + 1) * 128, :])
                    nc.gpsimd.dma_start(bk_sb[:, hh, :], bkd[hh * 128 : (hh + 1) * 128, :])
                nc.gpsimd.dma_start(onk_sb[:], onkd[:])
                nc.gpsimd.dma_start(negm_sb[:], negd[:])
                nc.gpsimd.dma_start(idnt_sb[:], idnd[:])
            for t in range(16):
                (nc.sync if t % 2 == 0 else nc.gpsimd).dma_start(
                    xb_sb[:, t, :], xbd[t * 128 : (t + 1) * 128, s0 : s0 + S]
                )
                if b == 0:
                    nc.sync.dma_start(wv_sb[:, t, :], wvd[t * 128 : (t + 1) * 128, :])
            if b == 0:
                for hh in range(HPC):
                    nc.gpsimd.dma_start(wo_sb[:, hh, :], wod[hh * 128 : (hh + 1) * 128, :])

            # --- q/k projections: fp8 DoubleRow, contract E in 8 pairs ---
            for m in range(HPC):
                for j in range(4):
                    js = slice(j * 512, (j + 1) * 512)
                    q_ps = projps()
                    for tp in range(8):
                        nc.tensor.matmul(
                            q_ps[:],
                            wq8_sb[:, 2 * tp : 2 * tp + 2, m * 128 : (m + 1) * 128],
                            x8_sb[:, 2 * tp : 2 * tp + 2, js],
                            start=(tp == 0),
                            stop=(tp == 7),
                            perf_mode=DR,
                        )
                    nc.vector.tensor_scalar_add(qT_sb[:, m, js], q_ps[:], bq_sb[:, m, :])
                    k_ps = projps()
                    for tp in range(8):
                        nc.tensor.matmul(
                            k_ps[:],
                            wk8_sb[:, 2 * tp : 2 * tp + 2, m * 128 : (m + 1) * 128],
                            x8_sb[:, 2 * tp : 2 * tp + 2, js],
                            start=(tp == 0),
                            stop=(tp == 7),
                            perf_mode=DR,
                        )
                    nc.scalar.activation(
                        kT_sb[:, m, js], k_ps[:], AF.Identity, bias=bk_sb[:, m, :]
                    )

            # --- v projection: bf16, x chunks stationary ---
            for si in range(16):
                v_ps = projps()
                for t in range(16):
                    nc.tensor.matmul(
                        v_ps[:, :DC],
                        xb_sb[:, t, si * 128 : (si + 1) * 128],
                        wv_sb[:, t, :],
                        start=(t == 0),
                        stop=(t == 15),
                    )
                nc.vector.tensor_copy(v_sb[:, si, :], v_ps[:, :DC])

            def outproj_group(qcs, interleaved):
                for qc in qcs:
                    y_t = yp.tile([128, 4, 512], BF, bufs=4, name="y_t")
                    for eb in range(4):
                        if interleaved:
                            y_ps = ps.tile([128, 512], F32, tag="yo", bufs=1, name="pj_yo")
                        else:
                            y_ps = projps()
                        nc.tensor.matmul(
                            y_ps[:],
                            ctxN_sb[:, 0, qc * 128 : (qc + 1) * 128],
                            wo_sb[:, 0, eb * 512 : (eb + 1) * 512],
                            start=True,
                            stop=False,
                        )
                        nc.tensor.matmul(
                            y_ps[:],
                            ctxN_sb[:, 1, qc * 128 : (qc + 1) * 128],
                            wo_sb[:, 1, eb * 512 : (eb + 1) * 512],
                            start=False,
                            stop=True,
                        )
                        if interleaved or (qc * 4 + eb) % 2 == 0:
                            nc.vector.tensor_copy(y_t[:, eb, :], y_ps[:])
                        else:
                            nc.scalar.copy(y_t[:, eb, :], y_ps[:])
                    nc.gpsimd.dma_start(
                        yd[s0 + qc * 128 : s0 + (qc + 1) * 128, :],
                        y_t[:].rearrange("p a b -> p (a b)"),
                    )

            # --- causal attention, scores kept transposed [k, q] ---
            LAG = 3
            for h in range(HPC):
                hd = slice(h * 128, (h + 1) * 128)
                for qb in range(4):
                    qs = slice(qb * 512, (qb + 1) * 512)
                    kmax = 4 * qb + 4
                    offs = [128 * (kc - 4 * qb) if kc >= 4 * qb else 0 for kc in range(kmax)]
                    pts = [None] * kmax
                    ctx_ps = ps.tile([128, 512], F32, tag="cx", bufs=2)
                    den_ps = ps.tile([128, 512], F32, tag="dn", bufs=2)
                    # interleave den/ctx accumulation (lagging LAG tiles)
                    # between score matmuls so PE never waits on ACT exp
                    for kc in range(kmax + LAG):
                        if kc < kmax:
                            off = offs[kc]
                            diag = kc >= 4 * qb
                            sc_ps = ps.tile([128, 512], F32, tag="sc", bufs=4)
                            nc.tensor.matmul(
                                sc_ps[:, off:],
                                kT_sb[:, h, kc * 128 : (kc + 1) * 128],
                                qT_sb[:, h, qb * 512 + off : (qb + 1) * 512],
                                start=True,
                                stop=not diag,
                            )
                            if diag:
                                # causal mask: only the leading 128 cols of a
                                # diag tile are triangular; add -1e9 there
                                nc.tensor.matmul(
                                    sc_ps[:, off : off + 128],
                                    idnt_sb[:],
                                    negm_sb[:],
                                    start=False,
                                    stop=True,
                                )
                            p_t = pp.tile([128, 512], BF)
                            nc.scalar.activation(
                                p_t[:, off:], sc_ps[:, off:], AF.Exp, scale=EXP_SCALE
                            )
                            pts[kc] = p_t
                        j = kc - LAG
                        if 0 <= j < kmax:
                            joff = offs[j]
                            nc.tensor.matmul(
                                den_ps[:, joff:],
                                onk_sb[:],
                                pts[j][:, joff:],
                                start=(j == 0),
                                stop=(j == kmax - 1),
                            )
                            nc.tensor.matmul(
                                ctx_ps[:, joff:],
                                v_sb[:, j, hd],
                                pts[j][:, joff:],
                                start=(j == 0),
                                stop=(j == kmax - 1),
                            )
                    lnd = dp.tile([128, 512], F32, tag="lnd", bufs=2)
                    nc.scalar.activation(lnd[:], den_ps[:], AF.Ln)
                    recb = dp.tile([128, 512], F32, tag="recb", bufs=2)
                    nc.scalar.activation(recb[:], lnd[:], AF.Exp, scale=-1.0)
                    nc.vector.tensor_tensor(
                        ctxN_sb[:, h, qs], ctx_ps[:], recb[:], ALU.mult
                    )
            outproj_group(range(16), interleaved=False)
    _split_excess_waits(nc)
    return nc


def _negm_np():
    kk = np.arange(128)[:, None]
    qq = np.arange(128)[None, :]
    return np.where(kk > qq, -1.0e9, 0.0).astype(bf16)


def kernel(**inputs):
    global LAST_RESULTS, _NC_CACHE
    x = np.asarray(inputs["x"], np.float32)
    Wq = np.asarray(inputs["Wq"], np.float32)
    bq = np.asarray(inputs["bq"], np.float32)
    Wk = np.asarray(inputs["Wk"], np.float32)
    bk = np.asarray(inputs["bk"], np.float32)
    Wv = np.asarray(inputs["Wv"], np.float32)
    bv = np.asarray(inputs["bv"], np.float32)
    Wo = np.asarray(inputs["Wo"], np.float32)
    bo = np.asarray(inputs["bo"], np.float32)

    xT = np.ascontiguousarray(x.reshape(BS, E).T)
    x8 = xT.astype(f8e4)
    xb = xT.astype(bf16)
    negm = _negm_np()
    ident = np.eye(128, dtype=bf16)
    onesk = np.ones((128, 128), bf16)

    in_maps = []
    for c in range(NCORES):
        dsl = slice(c * DC, (c + 1) * DC)
        in_maps.append(
            {
                "x8": x8,
                "xb": xb,
                "wq8": np.ascontiguousarray(Wq[dsl].T * AQ).astype(f8e4),
                "wk8": np.ascontiguousarray(Wk[dsl].T * AK).astype(f8e4),
                "wvT": np.ascontiguousarray(Wv[dsl].T).astype(bf16),
                "woT": np.ascontiguousarray(Wo[:, dsl].T).astype(bf16),
                "bq": np.ascontiguousarray((bq[dsl] * AQ).reshape(DC, 1)),
                "bk": np.ascontiguousarray((bk[dsl] * AK).reshape(DC, 1)),
                "negm": negm,
                "ident": ident,
                "onesk": onesk,
            }
        )

    if _NC_CACHE is None:
        _NC_CACHE = _build()
    res = run_bass_kernel_spmd(_NC_CACHE, in_maps, core_ids=list(range(NCORES)), trace=TRACE)
    LAST_RESULTS = res

    acc = None
    for r in res.results:
        yc = np.asarray(r["y"], np.float32)
        acc = yc if acc is None else acc + yc
    bo_eff = bo + bv @ Wo.T
    acc += bo_eff[None, :]
    return acc.reshape(B, S, E).astype(np.float32)


# revision 14
# speedup vs baseline: 1.3638x; 1.1228x over previous
import sys

sys.path.insert(0, "/opt/trn_rl_repo")

from contextlib import ExitStack

import ml_dtypes
import numpy as np

from concourse import bass, mybir, tile
from concourse.bass_utils import run_bass_kernel_spmd
from concourse.vector_clock import ScopedClock


def _patched_drain_and_barrier(self, tick_clock, wait_clock):
    # Workaround: this compiler rejects a drain carrying >1 sem wait
    # ([NCC_INLA001]); split extra waits onto single-wait nops.
    drain_inst = self.nc.sync.drain()
    wait_clock.add_sem_waits(
        drain_inst.ins, ScopedClock({None: tick_clock.global_clock})
    )
    si = drain_inst.ins.sync_info
    waits = list(si.on_wait) if si and si.on_wait else []
    if len(waits) > 1:
        drain_inst.ins.sync_info = mybir.SyncInfo(
            on_wait=[waits[0]], on_update=list(si.on_update or [])
        )
        for w in waits[1:]:
            nop = self.nc.sync.nop(nofuse=True)
            nop.ins.sync_info = mybir.SyncInfo(on_wait=[w], on_update=[])
    self.nc.all_engine_barrier()
    popped = self.nc._tile_sem_poison_stack.pop()
    assert popped is self._sem_poison
    self.nc.clear_and_free_semaphores(list(self.sems.allocated().values()))
    self.nc.all_engine_barrier()


tile.TileContext._drain_and_barrier = _patched_drain_and_barrier


def _split_excess_waits(nc, limit=1):
    # Workaround: this compiler allows only one sem wait on several
    # instruction encodings; move extra waits onto same-engine nops.
    eng_map = {
        mybir.EngineType.PE: nc.tensor,
        mybir.EngineType.Activation: nc.scalar,
        mybir.EngineType.DVE: nc.vector,
        mybir.EngineType.Pool: nc.gpsimd,
        mybir.EngineType.SP: nc.sync,
    }
    for blk in nc.cur_f.blocks:
        orig = list(blk.instructions)
        out = []
        for ins in orig:
            si = ins.sync_info
            waits = list(si.on_wait) if si and si.on_wait else []
            eng = eng_map.get(ins.engine)
            if len(waits) > limit and eng is not None:
                extra, keep = waits[:-limit], waits[-limit:]
                for w in extra:
                    nop = eng.nop(nofuse=True).ins
                    tail = nc.cur_f.blocks[-1].instructions
                    assert tail[-1] is nop
                    tail.pop()
                    nop.sync_info = mybir.SyncInfo(on_wait=[w], on_update=[])
                    out.append(nop)
                ins.sync_info = mybir.SyncInfo(
                    on_wait=keep, on_update=list(si.on_update or [])
                )
            out.append(ins)
        blk.instructions[:] = out


bf16 = ml_dtypes.bfloat16
f8e4 = ml_dtypes.float8_e4m3
BF = bass.mybir.dt.bfloat16
F8 = bass.mybir.dt.float8e4
F32 = bass.mybir.dt.float32
AF = mybir.ActivationFunctionType
ALU = mybir.AluOpType
DR = mybir.MatmulPerfMode.DoubleRow

B, S, E, H, D = 2, 2048, 2048, 16, 128
BS = B * S
NCORES = 8
HPC = H // NCORES  # heads per core
DC = HPC * D  # per-core head-dim width (256)
SCALE = 1.0 / float(np.sqrt(D))
AQ = SCALE * 128.0  # scale folded into Wq/bq so fp8 range is healthy
AK = 64.0  # scale folded into Wk/bk
EXP_SCALE = 1.0 / (128.0 * 64.0)

TRACE = False
LAST_RESULTS = None
_NC_CACHE = None


def _build():
    nc = bass.Bass()
    x8d = nc.declare_dram_parameter("x8", (E, BS), F8, isOutput=False)
    xbd = nc.declare_dram_parameter("xb", (E, BS), BF, isOutput=False)
    wq8d = nc.declare_dram_parameter("wq8", (E, DC), F8, isOutput=False)
    wk8d = nc.declare_dram_parameter("wk8", (E, DC), F8, isOutput=False)
    wvd = nc.declare_dram_parameter("wvT", (E, DC), BF, isOutput=False)
    wod = nc.declare_dram_parameter("woT", (DC, E), BF, isOutput=False)
    bqd = nc.declare_dram_parameter("bq", (DC, 1), F32, isOutput=False)
    bkd = nc.declare_dram_parameter("bk", (DC, 1), F32, isOutput=False)
    negd = nc.declare_dram_parameter("negm", (128, 128), BF, isOutput=False)
    negfd = nc.declare_dram_parameter("negf", (128, 128), BF, isOutput=False)
    idnd = nc.declare_dram_parameter("ident", (128, 128), BF, isOutput=False)
    onkd = nc.declare_dram_parameter("onesk", (128, 128), BF, isOutput=False)
    yd = nc.declare_dram_parameter("y", (BS, E), BF, isOutput=True)

    with ExitStack() as ctx:
        tc = ctx.enter_context(tile.TileContext(nc))
        wp = ctx.enter_context(tc.tile_pool(name="wp", bufs=1))
        xp = ctx.enter_context(tc.tile_pool(name="xp", bufs=1))
        bp = ctx.enter_context(tc.tile_pool(name="bp", bufs=1))
        pp = ctx.enter_context(tc.tile_pool(name="pp", bufs=8))
        ap_ = ctx.enter_context(tc.tile_pool(name="ap", bufs=2))
        dp = ctx.enter_context(tc.tile_pool(name="dp", bufs=2))
        yp = ctx.enter_context(tc.tile_pool(name="yp", bufs=8))
        ps = ctx.enter_context(tc.tile_pool(name="ps", bufs=1, space="PSUM"))

        # round-robin PSUM allocator for projection/output phases; the "sc"
        # tag holds 2-bank pair tiles (shared with attention), handed out in
        # halves, so all 8 banks stay uniformly shaped per tag
        _prr = [("cx", 2), ("dn", 2)]
        _pctr = [0]
        _scpair = [None, 0]

        def projps():
            k = _pctr[0] % 4
            _pctr[0] += 1
            if k < 2:
                tag, bufs = _prr[k]
                return ps.tile([128, 512], F32, tag=tag, bufs=bufs, name=f"pj_{tag}")
            if _scpair[1] == 0:
                _scpair[0] = ps.tile([128, 2, 512], F32, tag="sc", bufs=2, name="pj_sc")
            t = _scpair[0][:, _scpair[1], :]
            _scpair[1] ^= 1
            return t

        wq8_sb = wp.tile([128, 16, DC], F8)
        wk8_sb = wp.tile([128, 16, DC], F8)
        wv_sb = wp.tile([128, 16, DC], BF)
        wo_sb = wp.tile([128, HPC, E], BF)
        bq_sb = wp.tile([128, HPC, 1], F32)
        bk_sb = wp.tile([128, HPC, 1], F32)
        negm_sb = wp.tile([128, 128], BF)
        negf_sb = wp.tile([128, 128], BF)
        idnt_sb = wp.tile([128, 128], BF)
        onk_sb = wp.tile([128, 128], BF)

        x8_sb = xp.tile([128, 16, S], F8)
        xb_sb = xp.tile([128, 16, S], BF)
        qT_sb = bp.tile([128, HPC, S], BF)
        kT_sb = bp.tile([128, HPC, S], BF)
        v_sb = bp.tile([128, 16, DC], BF)
        ctxN_sb = bp.tile([128, HPC, S], BF)

        for b in range(B):
            s0 = b * S
            # --- input loads (b=0: weights interleaved for fast start) ---
            # spread DMA dispatch across engine queues: SP paces x8 (the
            # critical path), Pool paces weights, both feed the first chains
            for t in range(16):
                nc.sync.dma_start(x8_sb[:, t, :], x8d[t * 128 : (t + 1) * 128, s0 : s0 + S])
                if b == 0:
                    nc.gpsimd.dma_start(wq8_sb[:, t, :], wq8d[t * 128 : (t + 1) * 128, :])
                    nc.gpsimd.dma_start(wk8_sb[:, t, :], wk8d[t * 128 : (t + 1) * 128, :])
            if b == 0:
                for hh in range(HPC):
                    nc.gpsimd.dma_start(bq_sb[:, hh, :], bqd[hh * 128 : (hh + 1) * 128, :])
                    nc.gpsimd.dma_start(bk_sb[:, hh, :], bkd[hh * 128 : (hh + 1) * 128, :])
                nc.gpsimd.dma_start(onk_sb[:], onkd[:])
                nc.gpsimd.dma_start(negm_sb[:], negd[:])
                nc.gpsimd.dma_start(negf_sb[:], negfd[:])
                nc.gpsimd.dma_start(idnt_sb[:], idnd[:])
            for t in range(16):
                (nc.sync if t % 2 == 0 else nc.gpsimd).dma_start(
                    xb_sb[:, t, :], xbd[t * 128 : (t + 1) * 128, s0 : s0 + S]
                )
                if b == 0:
                    nc.sync.dma_start(wv_sb[:, t, :], wvd[t * 128 : (t + 1) * 128, :])
            if b == 0:
                for hh in range(HPC):
                    nc.gpsimd.dma_start(wo_sb[:, hh, :], wod[hh * 128 : (hh + 1) * 128, :])

            # --- q/k projections: fp8 DoubleRow, contract E in 8 pairs ---
            for m in range(HPC):
                for j in range(4):
                    js = slice(j * 512, (j + 1) * 512)
                    q_ps = projps()
                    for tp in range(8):
                        nc.tensor.matmul(
                            q_ps[:],
                            wq8_sb[:, 2 * tp : 2 * tp + 2, m * 128 : (m + 1) * 128],
                            x8_sb[:, 2 * tp : 2 * tp + 2, js],
                            start=(tp == 0),
                            stop=(tp == 7),
                            perf_mode=DR,
                        )
                    nc.scalar.activation(
                        qT_sb[:, m, js], q_ps[:], AF.Identity, bias=bq_sb[:, m, :]
                    )
                    k_ps = projps()
                    for tp in range(8):
                        nc.tensor.matmul(
                            k_ps[:],
                            wk8_sb[:, 2 * tp : 2 * tp + 2, m * 128 : (m + 1) * 128],
                            x8_sb[:, 2 * tp : 2 * tp + 2, js],
                            start=(tp == 0),
                            stop=(tp == 7),
                            perf_mode=DR,
                        )
                    nc.scalar.activation(
                        kT_sb[:, m, js], k_ps[:], AF.Identity, bias=bk_sb[:, m, :]
                    )

            # --- v projection: bf16, x chunks stationary ---
            for si in range(16):
                v_ps = projps()
                for t in range(16):
                    nc.tensor.matmul(
                        v_ps[:, :DC],
                        xb_sb[:, t, si * 128 : (si + 1) * 128],
                        wv_sb[:, t, :],
                        start=(t == 0),
                        stop=(t == 15),
                    )
                nc.vector.tensor_copy(v_sb[:, si, :], v_ps[:, :DC])

            def outproj_group(qcs, interleaved):
                for qc in qcs:
                    y_t = yp.tile([128, 4, 512], BF, bufs=4, name="y_t")
                    for eb in range(4):
                        if interleaved:
                            y_ps = ps.tile([128, 512], F32, tag="yo", bufs=1, name="pj_yo")
                        else:
                            y_ps = projps()
                        nc.tensor.matmul(
                            y_ps[:],
                            ctxN_sb[:, 0, qc * 128 : (qc + 1) * 128],
                            wo_sb[:, 0, eb * 512 : (eb + 1) * 512],
                            start=True,
                            stop=False,
                        )
                        nc.tensor.matmul(
                            y_ps[:],
                            ctxN_sb[:, 1, qc * 128 : (qc + 1) * 128],
                            wo_sb[:, 1, eb * 512 : (eb + 1) * 512],
                            start=False,
                            stop=True,
                        )
                        if interleaved or (qc * 4 + eb) % 2 == 0:
                            nc.vector.tensor_copy(y_t[:, eb, :], y_ps[:])
                        else:
                            nc.scalar.copy(y_t[:, eb, :], y_ps[:])
                    nc.gpsimd.dma_start(
                        yd[s0 + qc * 128 : s0 + (qc + 1) * 128, :],
                        y_t[:].rearrange("p a b -> p (a b)"),
                    )

            # --- causal attention, scores kept transposed [k, q] ---
            LAG = 3
            for h in range(HPC):
                hd = slice(h * 128, (h + 1) * 128)
                for qb in range(4):
                    qs = slice(qb * 512, (qb + 1) * 512)
                    kmax = 4 * qb + 4
                    offs = [128 * (kc - 4 * qb) if kc >= 4 * qb else 0 for kc in range(kmax)]
                    pts = [None] * kmax
                    psl = [None] * kmax
                    ctx_ps = ps.tile([128, 512], F32, tag="cx", bufs=2)
                    den_ps = ps.tile([128, 512], F32, tag="dn", bufs=2)
                    # interleave den/ctx accumulation (lagging LAG tiles)
                    # between score matmuls so PE never waits on ACT exp;
                    # score tiles come in 2-bank pairs with ONE exp per pair
                    sc_pair = None
                    for kc in range(kmax + LAG):
                        if kc < kmax:
                            off = offs[kc]
                            diag = kc >= 4 * qb
                            half = kc % 2
                            if half == 0:
                                sc_pair = ps.tile([128, 2, 512], F32, tag="sc", bufs=2)
                            nc.tensor.matmul(
                                sc_pair[:, half, off:],
                                kT_sb[:, h, kc * 128 : (kc + 1) * 128],
                                qT_sb[:, h, qb * 512 + off : (qb + 1) * 512],
                                start=True,
                                stop=not diag,
                            )
                            if diag:
                                # causal mask: only the leading 128 cols of a
                                # diag tile are triangular; add -1e9 there
                                nc.tensor.matmul(
                                    sc_pair[:, half, off : off + 128],
                                    idnt_sb[:],
                                    negm_sb[:],
                                    start=False,
                                    stop=True,
                                )
                            if half == 1:
                                off0 = offs[kc - 1]
                                if off != off0:
                                    # odd diag half starts 128 later: fill the
                                    # gap with -1e9 so the pair exp sees zeros
                                    nc.tensor.matmul(
                                        sc_pair[:, 1, off0:off],
                                        idnt_sb[:],
                                        negf_sb[:],
                                        start=True,
                                        stop=True,
                                    )
                                p_t = pp.tile([128, 2, 512], BF, bufs=6)
                                nc.scalar.activation(
                                    p_t[:, :, off0:],
                                    sc_pair[:, :, off0:],
                                    AF.Exp,
                                    scale=EXP_SCALE,
                                )
                                pts[kc - 1] = p_t
                                psl[kc - 1] = 0
                                pts[kc] = p_t
                                psl[kc] = 1
                        j = kc - LAG
                        if 0 <= j < kmax:
                            joff = offs[j]
                            nc.tensor.matmul(
                                den_ps[:, joff:],
                                onk_sb[:],
                                pts[j][:, psl[j], joff:],
                                start=(j == 0),
                                stop=(j == kmax - 1),
                            )
                            nc.tensor.matmul(
                                ctx_ps[:, joff:],
                                v_sb[:, j, hd],
                                pts[j][:, psl[j], joff:],
                                start=(j == 0),
                                stop=(j == kmax - 1),
                            )
                    lnd = dp.tile([128, 512], F32, tag="lnd", bufs=2)
                    nc.scalar.activation(lnd[:], den_ps[:], AF.Ln)
                    recb = dp.tile([128, 512], F32, tag="recb", bufs=2)
                    nc.scalar.activation(recb[:], lnd[:], AF.Exp, scale=-1.0)
                    nc.vector.tensor_tensor(
                        ctxN_sb[:, h, qs], ctx_ps[:], recb[:], ALU.mult
                    )
            outproj_group(range(16), interleaved=False)
    _split_excess_waits(nc)
    return nc


def _negm_np():
    kk = np.arange(128)[:, None]
    qq = np.arange(128)[None, :]
    return np.where(kk > qq, -1.0e9, 0.0).astype(bf16)


def kernel(**inputs):
    global LAST_RESULTS, _NC_CACHE
    x = np.asarray(inputs["x"], np.float32)
    Wq = np.asarray(inputs["Wq"], np.float32)
    bq = np.asarray(inputs["bq"], np.float32)
    Wk = np.asarray(inputs["Wk"], np.float32)
    bk = np.asarray(inputs["bk"], np.float32)
    Wv = np.asarray(inputs["Wv"], np.float32)
    bv = np.asarray(inputs["bv"], np.float32)
    Wo = np.asarray(inputs["Wo"], np.float32)
    bo = np.asarray(inputs["bo"], np.float32)

    xT = np.ascontiguousarray(x.reshape(BS, E).T)
    x8 = xT.astype(f8e4)
    xb = xT.astype(bf16)
    negm = _negm_np()
    negf = np.full((128, 128), -1.0e9, dtype=bf16)
    ident = np.eye(128, dtype=bf16)
    onesk = np.ones((128, 128), bf16)

    in_maps = []
    for c in range(NCORES):
        dsl = slice(c * DC, (c + 1) * DC)
        in_maps.append(
            {
                "x8": x8,
                "xb": xb,
                "wq8": np.ascontiguousarray(Wq[dsl].T * AQ).astype(f8e4),
                "wk8": np.ascontiguousarray(Wk[dsl].T * AK).astype(f8e4),
                "wvT": np.ascontiguousarray(Wv[dsl].T).astype(bf16),
                "woT": np.ascontiguousarray(Wo[:, dsl].T).astype(bf16),
                "bq": np.ascontiguousarray((bq[dsl] * AQ).reshape(DC, 1)),
                "bk": np.ascontiguousarray((bk[dsl] * AK).reshape(DC, 1)),
                "negm": negm,
                "negf": negf,
                "ident": ident,
                "onesk": onesk,
            }
        )

    if _NC_CACHE is None:
        _NC_CACHE = _build()
    res = run_bass_kernel_spmd(_NC_CACHE, in_maps, core_ids=list(range(NCORES)), trace=TRACE)
    LAST_RESULTS = res

    acc = None
    for r in res.results:
        yc = np.asarray(r["y"], np.float32)
        acc = yc if acc is None else acc + yc
    bo_eff = bo + bv @ Wo.T
    acc += bo_eff[None, :]
    return acc.reshape(B, S, E).astype(np.float32)


# revision 15
# speedup vs baseline: 1.4406x; 1.0563x over previous
import sys

sys.path.insert(0, "/opt/trn_rl_repo")

from contextlib import ExitStack

import ml_dtypes
import numpy as np

from concourse import bass, mybir, tile
from concourse.bass_utils import run_bass_kernel_spmd
from concourse.vector_clock import ScopedClock


def _patched_drain_and_barrier(self, tick_clock, wait_clock):
    # Workaround: this compiler rejects a drain carrying >1 sem wait
    # ([NCC_INLA001]); split extra waits onto single-wait nops.
    drain_inst = self.nc.sync.drain()
    wait_clock.add_sem_waits(
        drain_inst.ins, ScopedClock({None: tick_clock.global_clock})
    )
    si = drain_inst.ins.sync_info
    waits = list(si.on_wait) if si and si.on_wait else []
    if len(waits) > 1:
        drain_inst.ins.sync_info = mybir.SyncInfo(
            on_wait=[waits[0]], on_update=list(si.on_update or [])
        )
        for w in waits[1:]:
            nop = self.nc.sync.nop(nofuse=True)
            nop.ins.sync_info = mybir.SyncInfo(on_wait=[w], on_update=[])
    self.nc.all_engine_barrier()
    popped = self.nc._tile_sem_poison_stack.pop()
    assert popped is self._sem_poison
    self.nc.clear_and_free_semaphores(list(self.sems.allocated().values()))
    self.nc.all_engine_barrier()


tile.TileContext._drain_and_barrier = _patched_drain_and_barrier


def _split_excess_waits(nc, limit=1):
    # Workaround: this compiler allows only one sem wait on several
    # instruction encodings; move extra waits onto same-engine nops.
    eng_map = {
        mybir.EngineType.PE: nc.tensor,
        mybir.EngineType.Activation: nc.scalar,
        mybir.EngineType.DVE: nc.vector,
        mybir.EngineType.Pool: nc.gpsimd,
        mybir.EngineType.SP: nc.sync,
    }
    for blk in nc.cur_f.blocks:
        orig = list(blk.instructions)
        out = []
        for ins in orig:
            si = ins.sync_info
            waits = list(si.on_wait) if si and si.on_wait else []
            eng = eng_map.get(ins.engine)
            if len(waits) > limit and eng is not None:
                extra, keep = waits[:-limit], waits[-limit:]
                for w in extra:
                    nop = eng.nop(nofuse=True).ins
                    tail = nc.cur_f.blocks[-1].instructions
                    assert tail[-1] is nop
                    tail.pop()
                    nop.sync_info = mybir.SyncInfo(on_wait=[w], on_update=[])
                    out.append(nop)
                ins.sync_info = mybir.SyncInfo(
                    on_wait=keep, on_update=list(si.on_update or [])
                )
            out.append(ins)
        blk.instructions[:] = out


bf16 = ml_dtypes.bfloat16
f8e4 = ml_dtypes.float8_e4m3
BF = bass.mybir.dt.bfloat16
F8 = bass.mybir.dt.float8e4
F32 = bass.mybir.dt.float32
AF = mybir.ActivationFunctionType
ALU = mybir.AluOpType
DR = mybir.MatmulPerfMode.DoubleRow

B, S, E, H, D = 2, 2048, 2048, 16, 128
BS = B * S
NCORES = 8
HPC = H // NCORES  # heads per core
DC = HPC * D  # per-core head-dim width (256)
SCALE = 1.0 / float(np.sqrt(D))
AQ = SCALE * 128.0  # scale folded into Wq/bq so fp8 range is healthy
AK = 64.0  # scale folded into Wk/bk
EXP_SCALE = 1.0 / (128.0 * 64.0)

TRACE = False
LAST_RESULTS = None
_NC_CACHE = None


def _build():
    nc = bass.Bass()
    x8d = nc.declare_dram_parameter("x8", (E, BS), F8, isOutput=False)
    xbd = nc.declare_dram_parameter("xb", (E, BS), BF, isOutput=False)
    wq8d = nc.declare_dram_parameter("wq8", (E, DC), F8, isOutput=False)
    wk8d = nc.declare_dram_parameter("wk8", (E, DC), F8, isOutput=False)
    wvd = nc.declare_dram_parameter("wvT", (E, DC), BF, isOutput=False)
    wod = nc.declare_dram_parameter("woT", (DC, E), BF, isOutput=False)
    bqd = nc.declare_dram_parameter("bq", (DC, 1), F32, isOutput=False)
    bkd = nc.declare_dram_parameter("bk", (DC, 1), F32, isOutput=False)
    negd = nc.declare_dram_parameter("negm", (128, 128), BF, isOutput=False)
    negfd = nc.declare_dram_parameter("negf", (128, 128), BF, isOutput=False)
    idnd = nc.declare_dram_parameter("ident", (128, 128), BF, isOutput=False)
    onkd = nc.declare_dram_parameter("onesk", (128, 128), BF, isOutput=False)
    yd = nc.declare_dram_parameter("y", (BS, E), BF, isOutput=True)

    with ExitStack() as ctx:
        tc = ctx.enter_context(tile.TileContext(nc))
        wp = ctx.enter_context(tc.tile_pool(name="wp", bufs=1))
        xp = ctx.enter_context(tc.tile_pool(name="xp", bufs=1))
        bp = ctx.enter_context(tc.tile_pool(name="bp", bufs=1))
        pp = ctx.enter_context(tc.tile_pool(name="pp", bufs=8))
        ap_ = ctx.enter_context(tc.tile_pool(name="ap", bufs=2))
        dp = ctx.enter_context(tc.tile_pool(name="dp", bufs=2))
        yp = ctx.enter_context(tc.tile_pool(name="yp", bufs=8))
        ps = ctx.enter_context(tc.tile_pool(name="ps", bufs=1, space="PSUM"))

        # round-robin PSUM tag allocator for projection/output phases
        _prr = [("sc", 4), ("cx", 2), ("dn", 2)]
        _pctr = [0]

        def projps():
            tag, bufs = _prr[_pctr[0] % 3]
            _pctr[0] += 1
            return ps.tile([128, 512], F32, tag=tag, bufs=bufs, name=f"pj_{tag}")

        wq8_sb = wp.tile([128, 16, DC], F8)
        wk8_sb = wp.tile([128, 16, DC], F8)
        wv_sb = wp.tile([128, 16, DC], BF)
        wo_sb = wp.tile([128, HPC, E], BF)
        bq_sb = wp.tile([128, HPC, 1], F32)
        bk_sb = wp.tile([128, HPC, 1], F32)
        negm_sb = wp.tile([128, 128], BF)
        negf_sb = wp.tile([128, 128], BF)
        idnt_sb = wp.tile([128, 128], BF)
        onk_sb = wp.tile([128, 128], BF)

        x8_sb = xp.tile([128, 16, S], F8)
        xb_sb = xp.tile([128, 16, S], BF)
        qT_sb = bp.tile([128, HPC, S], BF)
        kT_sb = bp.tile([128, HPC, S], BF)
        v_sb = bp.tile([128, 16, DC], BF)
        ctxN_sb = bp.tile([128, HPC, S], BF)

        for b in range(B):
            s0 = b * S
            # --- input loads (b=0: weights interleaved for fast start) ---
            # spread DMA dispatch across engine queues: SP paces x8 (the
            # critical path), Pool paces weights, both feed the first chains
            for t in range(16):
                nc.sync.dma_start(x8_sb[:, t, :], x8d[t * 128 : (t + 1) * 128, s0 : s0 + S])
                if b == 0:
                    nc.gpsimd.dma_start(wq8_sb[:, t, :], wq8d[t * 128 : (t + 1) * 128, :])
                    nc.gpsimd.dma_start(wk8_sb[:, t, :], wk8d[t * 128 : (t + 1) * 128, :])
            if b == 0:
                for hh in range(HPC):
                    nc.gpsimd.dma_start(bq_sb[:, hh, :], bqd[hh * 128 : (hh + 1) * 128, :])
                    nc.gpsimd.dma_start(bk_sb[:, hh, :], bkd[hh * 128 : (hh + 1) * 128, :])
                nc.gpsimd.dma_start(onk_sb[:], onkd[:])
                nc.gpsimd.dma_start(negm_sb[:], negd[:])
                nc.gpsimd.dma_start(negf_sb[:], negfd[:])
                nc.gpsimd.dma_start(idnt_sb[:], idnd[:])
            for t in range(16):
                (nc.sync if t % 2 == 0 else nc.gpsimd).dma_start(
                    xb_sb[:, t, :], xbd[t * 128 : (t + 1) * 128, s0 : s0 + S]
                )
                if b == 0:
                    nc.sync.dma_start(wv_sb[:, t, :], wvd[t * 128 : (t + 1) * 128, :])
            if b == 0:
                for hh in range(HPC):
                    nc.gpsimd.dma_start(wo_sb[:, hh, :], wod[hh * 128 : (hh + 1) * 128, :])

            # --- q/k projections: fp8 DoubleRow, contract E in 8 pairs ---
            for m in range(HPC):
                for j in range(4):
                    js = slice(j * 512, (j + 1) * 512)
                    q_ps = projps()
                    for tp in range(8):
                        nc.tensor.matmul(
                            q_ps[:],
                            wq8_sb[:, 2 * tp : 2 * tp + 2, m * 128 : (m + 1) * 128],
                            x8_sb[:, 2 * tp : 2 * tp + 2, js],
                            start=(tp == 0),
                            stop=(tp == 7),
                            perf_mode=DR,
                        )
                    nc.scalar.activation(
                        qT_sb[:, m, js], q_ps[:], AF.Identity, bias=bq_sb[:, m, :]
                    )
                    k_ps = projps()
                    for tp in range(8):
                        nc.tensor.matmul(
                            k_ps[:],
                            wk8_sb[:, 2 * tp : 2 * tp + 2, m * 128 : (m + 1) * 128],
                            x8_sb[:, 2 * tp : 2 * tp + 2, js],
                            start=(tp == 0),
                            stop=(tp == 7),
                            perf_mode=DR,
                        )
                    nc.scalar.activation(
                        kT_sb[:, m, js], k_ps[:], AF.Identity, bias=bk_sb[:, m, :]
                    )

            # --- v projection: bf16, x chunks stationary ---
            for si in range(16):
                v_ps = projps()
                for t in range(16):
                    nc.tensor.matmul(
                        v_ps[:, :DC],
                        xb_sb[:, t, si * 128 : (si + 1) * 128],
                        wv_sb[:, t, :],
                        start=(t == 0),
                        stop=(t == 15),
                    )
                nc.vector.tensor_copy(v_sb[:, si, :], v_ps[:, :DC])

            def outproj_group(qcs, interleaved):
                for qc in qcs:
                    y_t = yp.tile([128, 4, 512], BF, bufs=4, name="y_t")
                    for eb in range(4):
                        if interleaved:
                            y_ps = ps.tile([128, 512], F32, tag="yo", bufs=1, name="pj_yo")
                        else:
                            y_ps = projps()
                        nc.tensor.matmul(
                            y_ps[:],
                            ctxN_sb[:, 0, qc * 128 : (qc + 1) * 128],
                            wo_sb[:, 0, eb * 512 : (eb + 1) * 512],
                            start=True,
                            stop=False,
                        )
                        nc.tensor.matmul(
                            y_ps[:],
                            ctxN_sb[:, 1, qc * 128 : (qc + 1) * 128],
                            wo_sb[:, 1, eb * 512 : (eb + 1) * 512],
                            start=False,
                            stop=True,
                        )
                        if interleaved or (qc * 4 + eb) % 2 == 0:
                            nc.vector.tensor_copy(y_t[:, eb, :], y_ps[:])
                        else:
                            nc.scalar.copy(y_t[:, eb, :], y_ps[:])
                    nc.gpsimd.dma_start(
                        yd[s0 + qc * 128 : s0 + (qc + 1) * 128, :],
                        y_t[:].rearrange("p a b -> p (a b)"),
                    )

            # --- causal attention, scores kept transposed [k, q] ---
            LAG = 3
            for h in range(HPC):
                hd = slice(h * 128, (h + 1) * 128)
                for qb in range(4):
                    qs = slice(qb * 512, (qb + 1) * 512)
                    kmax = 4 * qb + 4
                    offs = [128 * (kc - 4 * qb) if kc >= 4 * qb else 0 for kc in range(kmax)]
                    pts = [None] * kmax
                    ctx_ps = ps.tile([128, 512], F32, tag="cx", bufs=2)
                    den_ps = ps.tile([128, 512], F32, tag="dn", bufs=2)
                    # interleave den/ctx accumulation (lagging LAG tiles)
                    # between score matmuls so PE never waits on ACT exp
                    for kc in range(kmax + LAG):
                        if kc < kmax:
                            off = offs[kc]
                            diag = kc >= 4 * qb
                            sc_ps = ps.tile([128, 512], F32, tag="sc", bufs=4)
                            nc.tensor.matmul(
                                sc_ps[:, off:],
                                kT_sb[:, h, kc * 128 : (kc + 1) * 128],
                                qT_sb[:, h, qb * 512 + off : (qb + 1) * 512],
                                start=True,
                                stop=not diag,
                            )
                            if diag:
                                # causal mask: only the leading 128 cols of a
                                # diag tile are triangular; add -1e9 there
                                nc.tensor.matmul(
                                    sc_ps[:, off : off + 128],
                                    idnt_sb[:],
                                    negm_sb[:],
                                    start=False,
                                    stop=True,
                                )
                            p_t = pp.tile([128, 512], BF)
                            nc.scalar.activation(
                                p_t[:, off:], sc_ps[:, off:], AF.Exp, scale=EXP_SCALE
                            )
                            pts[kc] = p_t
                        j = kc - LAG
                        if 0 <= j < kmax:
                            joff = offs[j]
                            nc.tensor.matmul(
                                den_ps[:, joff:],
                                onk_sb[:],
                                pts[j][:, joff:],
                                start=(j == 0),
                                stop=(j == kmax - 1),
                            )
                            nc.tensor.matmul(
                                ctx_ps[:, joff:],
                                v_sb[:, j, hd],
                                pts[j][:, joff:],
                                start=(j == 0),
                                stop=(j == kmax - 1),
                            )
                    lnd = dp.tile([128, 512], F32, tag="lnd", bufs=2)
                    nc.scalar.activation(lnd[:], den_ps[:], AF.Ln)
                    recb = dp.tile([128, 512], F32, tag="recb", bufs=2)
                    nc.scalar.activation(recb[:], lnd[:], AF.Exp, scale=-1.0)
                    nc.vector.tensor_tensor(
                        ctxN_sb[:, h, qs], ctx_ps[:], recb[:], ALU.mult
                    )
            outproj_group(range(16), interleaved=False)
    _split_excess_waits(nc)
    return nc


def _negm_np():
    kk = np.arange(128)[:, None]
    qq = np.arange(128)[None, :]
    return np.where(kk > qq, -1.0e9, 0.0).astype(bf16)


def kernel(**inputs):
    global LAST_RESULTS, _NC_CACHE
    x = np.asarray(inputs["x"], np.float32)
    Wq = np.asarray(inputs["Wq"], np.float32)
    bq = np.asarray(inputs["bq"], np.float32)
    Wk = np.asarray(inputs["Wk"], np.float32)
    bk = np.asarray(inputs["bk"], np.float32)
    Wv = np.asarray(inputs["Wv"], np.float32)
    bv = np.asarray(inputs["bv"], np.float32)
    Wo = np.asarray(inputs["Wo"], np.float32)
    bo = np.asarray(inputs["bo"], np.float32)

    xT = np.ascontiguousarray(x.reshape(BS, E).T)
    x8 = xT.astype(f8e4)
    xb = xT.astype(bf16)
    negm = _negm_np()
    negf = np.full((128, 128), -1.0e9, dtype=bf16)
    ident = np.eye(128, dtype=bf16)
    onesk = np.ones((128, 128), bf16)

    in_maps = []
    for c in range(NCORES):
        dsl = slice(c * DC, (c + 1) * DC)
        in_maps.append(
            {
                "x8": x8,
                "xb": xb,
                "wq8": np.ascontiguousarray(Wq[dsl].T * AQ).astype(f8e4),
                "wk8": np.ascontiguousarray(Wk[dsl].T * AK).astype(f8e4),
                "wvT": np.ascontiguousarray(Wv[dsl].T).astype(bf16),
                "woT": np.ascontiguousarray(Wo[:, dsl].T).astype(bf16),
                "bq": np.ascontiguousarray((bq[dsl] * AQ).reshape(DC, 1)),
                "bk": np.ascontiguousarray((bk[dsl] * AK).reshape(DC, 1)),
                "negm": negm,
                "negf": negf,
                "ident": ident,
                "onesk": onesk,
            }
        )

    if _NC_CACHE is None:
        _NC_CACHE = _build()
    res = run_bass_kernel_spmd(_NC_CACHE, in_maps, core_ids=list(range(NCORES)), trace=TRACE)
    LAST_RESULTS = res

    acc = None
    for r in res.results:
        yc = np.asarray(r["y"], np.float32)
        acc = yc if acc is None else acc + yc
    bo_eff = bo + bv @ Wo.T
    acc += bo_eff[None, :]
    return acc.reshape(B, S, E).astype(np.float32)


# revision 16
# speedup vs baseline: 1.4428x; 1.0015x over previous
import sys

sys.path.insert(0, "/opt/trn_rl_repo")

from contextlib import ExitStack

import ml_dtypes
import numpy as np

from concourse import bass, mybir, tile
from concourse.bass_utils import run_bass_kernel_spmd
from concourse.vector_clock import ScopedClock


def _patched_drain_and_barrier(self, tick_clock, wait_clock):
    # Workaround: this compiler rejects a drain carrying >1 sem wait
    # ([NCC_INLA001]); split extra waits onto single-wait nops.
    drain_inst = self.nc.sync.drain()
    wait_clock.add_sem_waits(
        drain_inst.ins, ScopedClock({None: tick_clock.global_clock})
    )
    si = drain_inst.ins.sync_info
    waits = list(si.on_wait) if si and si.on_wait else []
    if len(waits) > 1:
        drain_inst.ins.sync_info = mybir.SyncInfo(
            on_wait=[waits[0]], on_update=list(si.on_update or [])
        )
        for w in waits[1:]:
            nop = self.nc.sync.nop(nofuse=True)
            nop.ins.sync_info = mybir.SyncInfo(on_wait=[w], on_update=[])
    self.nc.all_engine_barrier()
    popped = self.nc._tile_sem_poison_stack.pop()
    assert popped is self._sem_poison
    self.nc.clear_and_free_semaphores(list(self.sems.allocated().values()))
    self.nc.all_engine_barrier()


tile.TileContext._drain_and_barrier = _patched_drain_and_barrier


def _split_excess_waits(nc, limit=1):
    # Workaround: this compiler allows only one sem wait on several
    # instruction encodings; move extra waits onto same-engine nops.
    eng_map = {
        mybir.EngineType.PE: nc.tensor,
        mybir.EngineType.Activation: nc.scalar,
        mybir.EngineType.DVE: nc.vector,
        mybir.EngineType.Pool: nc.gpsimd,
        mybir.EngineType.SP: nc.sync,
    }
    for blk in nc.cur_f.blocks:
        orig = list(blk.instructions)
        out = []
        for ins in orig:
            si = ins.sync_info
            waits = list(si.on_wait) if si and si.on_wait else []
            eng = eng_map.get(ins.engine)
            if len(waits) > limit and eng is not None:
                extra, keep = waits[:-limit], waits[-limit:]
                for w in extra:
                    nop = eng.nop(nofuse=True).ins
                    tail = nc.cur_f.blocks[-1].instructions
                    assert tail[-1] is nop
                    tail.pop()
                    nop.sync_info = mybir.SyncInfo(on_wait=[w], on_update=[])
                    out.append(nop)
                ins.sync_info = mybir.SyncInfo(
                    on_wait=keep, on_update=list(si.on_update or [])
                )
            out.append(ins)
        blk.instructions[:] = out


bf16 = ml_dtypes.bfloat16
f8e4 = ml_dtypes.float8_e4m3
BF = bass.mybir.dt.bfloat16
F8 = bass.mybir.dt.float8e4
F32 = bass.mybir.dt.float32
AF = mybir.ActivationFunctionType
ALU = mybir.AluOpType
DR = mybir.MatmulPerfMode.DoubleRow

B, S, E, H, D = 2, 2048, 2048, 16, 128
BS = B * S
NCORES = 8
HPC = H // NCORES  # heads per core
DC = HPC * D  # per-core head-dim width (256)
SCALE = 1.0 / float(np.sqrt(D))
AQ = SCALE * 128.0  # scale folded into Wq/bq so fp8 range is healthy
AK = 64.0  # scale folded into Wk/bk
EXP_SCALE = 1.0 / (128.0 * 64.0)

TRACE = False
LAST_RESULTS = None
_NC_CACHE = None


def _build():
    nc = bass.Bass()
    x8d = nc.declare_dram_parameter("x8", (E, BS), F8, isOutput=False)
    xbd = nc.declare_dram_parameter("xb", (E, BS), BF, isOutput=False)
    wq8d = nc.declare_dram_parameter("wq8", (E, DC), F8, isOutput=False)
    wk8d = nc.declare_dram_parameter("wk8", (E, DC), F8, isOutput=False)
    wvd = nc.declare_dram_parameter("wvT", (E, DC), BF, isOutput=False)
    wod = nc.declare_dram_parameter("woT", (DC, E), BF, isOutput=False)
    bqd = nc.declare_dram_parameter("bq", (DC, 1), F32, isOutput=False)
    bkd = nc.declare_dram_parameter("bk", (DC, 1), F32, isOutput=False)
    negd = nc.declare_dram_parameter("negm", (128, 128), BF, isOutput=False)
    negfd = nc.declare_dram_parameter("negf", (128, 128), BF, isOutput=False)
    idnd = nc.declare_dram_parameter("ident", (128, 128), BF, isOutput=False)
    onkd = nc.declare_dram_parameter("onesk", (128, 128), BF, isOutput=False)
    yd = nc.declare_dram_parameter("y", (BS, E), BF, isOutput=True)

    with ExitStack() as ctx:
        tc = ctx.enter_context(tile.TileContext(nc))
        wp = ctx.enter_context(tc.tile_pool(name="wp", bufs=1))
        xp = ctx.enter_context(tc.tile_pool(name="xp", bufs=1))
        bp = ctx.enter_context(tc.tile_pool(name="bp", bufs=1))
        pp = ctx.enter_context(tc.tile_pool(name="pp", bufs=8))
        ap_ = ctx.enter_context(tc.tile_pool(name="ap", bufs=2))
        dp = ctx.enter_context(tc.tile_pool(name="dp", bufs=2))
        yp = ctx.enter_context(tc.tile_pool(name="yp", bufs=8))
        ps = ctx.enter_context(tc.tile_pool(name="ps", bufs=1, space="PSUM"))

        # round-robin PSUM tag allocator for projection/output phases
        _prr = [("sc", 4), ("cx", 2), ("dn", 2)]
        _pctr = [0]

        def projps():
            tag, bufs = _prr[_pctr[0] % 3]
            _pctr[0] += 1
            return ps.tile([128, 512], F32, tag=tag, bufs=bufs, name=f"pj_{tag}")

        wq8_sb = wp.tile([128, 16, DC], F8)
        wk8_sb = wp.tile([128, 16, DC], F8)
        wv_sb = wp.tile([128, 16, DC], BF)
        wo_sb = wp.tile([128, HPC, E], BF)
        bq_sb = wp.tile([128, HPC, 1], F32)
        bk_sb = wp.tile([128, HPC, 1], F32)
        negm_sb = wp.tile([128, 128], BF)
        negf_sb = wp.tile([128, 128], BF)
        idnt_sb = wp.tile([128, 128], BF)
        onk_sb = wp.tile([128, 128], BF)

        x8_sb = xp.tile([128, 16, S], F8)
        xb_sb = xp.tile([128, 16, S], BF)
        qT_sb = bp.tile([128, HPC, S], BF)
        kT_sb = bp.tile([128, HPC, S], BF)
        v_sb = bp.tile([128, 16, DC], BF)
        ctxN_sb = bp.tile([128, HPC, S], BF)

        for b in range(B):
            s0 = b * S
            # --- input loads (b=0: weights interleaved for fast start) ---
            # spread DMA dispatch across engine queues: SP paces x8 (the
            # critical path), Pool paces weights, both feed the first chains
            for t in range(16):
                nc.sync.dma_start(x8_sb[:, t, :], x8d[t * 128 : (t + 1) * 128, s0 : s0 + S])
                if b == 0:
                    nc.gpsimd.dma_start(wq8_sb[:, t, :], wq8d[t * 128 : (t + 1) * 128, :])
                    nc.gpsimd.dma_start(wk8_sb[:, t, :], wk8d[t * 128 : (t + 1) * 128, :])
            if b == 0:
                for hh in range(HPC):
                    nc.gpsimd.dma_start(bq_sb[:, hh, :], bqd[hh * 128 : (hh + 1) * 128, :])
                    nc.gpsimd.dma_start(bk_sb[:, hh, :], bkd[hh * 128 : (hh + 1) * 128, :])
                nc.gpsimd.dma_start(onk_sb[:], onkd[:])
                nc.gpsimd.dma_start(negm_sb[:], negd[:])
                nc.gpsimd.dma_start(negf_sb[:], negfd[:])
                nc.gpsimd.dma_start(idnt_sb[:], idnd[:])
            for t in range(16):
                (nc.sync if t % 2 == 0 else nc.gpsimd).dma_start(
                    xb_sb[:, t, :], xbd[t * 128 : (t + 1) * 128, s0 : s0 + S]
                )
                if b == 0:
                    nc.sync.dma_start(wv_sb[:, t, :], wvd[t * 128 : (t + 1) * 128, :])
            if b == 0:
                for hh in range(HPC):
                    nc.gpsimd.dma_start(wo_sb[:, hh, :], wod[hh * 128 : (hh + 1) * 128, :])

            # --- q/k projections: fp8 DoubleRow, contract E in 8 pairs ---
            for m in range(HPC):
                for j in range(4):
                    js = slice(j * 512, (j + 1) * 512)
                    q_ps = projps()
                    for tp in range(8):
                        nc.tensor.matmul(
                            q_ps[:],
                            wq8_sb[:, 2 * tp : 2 * tp + 2, m * 128 : (m + 1) * 128],
                            x8_sb[:, 2 * tp : 2 * tp + 2, js],
                            start=(tp == 0),
                            stop=(tp == 7),
                            perf_mode=DR,
                        )
                    nc.scalar.activation(
                        qT_sb[:, m, js], q_ps[:], AF.Identity, bias=bq_sb[:, m, :]
                    )
                    k_ps = projps()
                    for tp in range(8):
                        nc.tensor.matmul(
                            k_ps[:],
                            wk8_sb[:, 2 * tp : 2 * tp + 2, m * 128 : (m + 1) * 128],
                            x8_sb[:, 2 * tp : 2 * tp + 2, js],
                            start=(tp == 0),
                            stop=(tp == 7),
                            perf_mode=DR,
                        )
                    nc.scalar.activation(
                        kT_sb[:, m, js], k_ps[:], AF.Identity, bias=bk_sb[:, m, :]
                    )

            # --- v projection: bf16, x chunks stationary ---
            for si in range(16):
                v_ps = projps()
                for t in range(16):
                    nc.tensor.matmul(
                        v_ps[:, :DC],
                        xb_sb[:, t, si * 128 : (si + 1) * 128],
                        wv_sb[:, t, :],
                        start=(t == 0),
                        stop=(t == 15),
                    )
                nc.vector.tensor_copy(v_sb[:, si, :], v_ps[:, :DC])

            def outproj_group(qcs, interleaved):
                for qc in qcs:
                    y_t = yp.tile([128, 4, 512], BF, bufs=4, name="y_t")
                    for eb in range(4):
                        if interleaved:
                            y_ps = ps.tile([128, 512], F32, tag="yo", bufs=1, name="pj_yo")
                        else:
                            y_ps = projps()
                        nc.tensor.matmul(
                            y_ps[:],
                            ctxN_sb[:, 0, qc * 128 : (qc + 1) * 128],
                            wo_sb[:, 0, eb * 512 : (eb + 1) * 512],
                            start=True,
                            stop=False,
                        )
                        nc.tensor.matmul(
                            y_ps[:],
                            ctxN_sb[:, 1, qc * 128 : (qc + 1) * 128],
                            wo_sb[:, 1, eb * 512 : (eb + 1) * 512],
                            start=False,
                            stop=True,
                        )
                        if interleaved or (qc * 4 + eb) % 2 == 0:
                            nc.vector.tensor_copy(y_t[:, eb, :], y_ps[:])
                        else:
                            nc.scalar.copy(y_t[:, eb, :], y_ps[:])
                    nc.gpsimd.dma_start(
                        yd[s0 + qc * 128 : s0 + (qc + 1) * 128, :],
                        y_t[:].rearrange("p a b -> p (a b)"),
                    )

            # --- causal attention, scores kept transposed [k, q] ---
            LAG = 4
            for h in range(HPC):
                hd = slice(h * 128, (h + 1) * 128)
                for qb in range(4):
                    qs = slice(qb * 512, (qb + 1) * 512)
                    kmax = 4 * qb + 4
                    offs = [128 * (kc - 4 * qb) if kc >= 4 * qb else 0 for kc in range(kmax)]
                    pts = [None] * kmax
                    ctx_ps = ps.tile([128, 512], F32, tag="cx", bufs=2)
                    den_ps = ps.tile([128, 512], F32, tag="dn", bufs=2)
                    # interleave den/ctx accumulation (lagging LAG tiles)
                    # between score matmuls so PE never waits on ACT exp
                    for kc in range(kmax + LAG):
                        if kc < kmax:
                            off = offs[kc]
                            diag = kc >= 4 * qb
                            sc_ps = ps.tile([128, 512], F32, tag="sc", bufs=4)
                            nc.tensor.matmul(
                                sc_ps[:, off:],
                                kT_sb[:, h, kc * 128 : (kc + 1) * 128],
                                qT_sb[:, h, qb * 512 + off : (qb + 1) * 512],
                                start=True,
                                stop=not diag,
                            )
                            if diag:
                                # causal mask: only the leading 128 cols of a
                                # diag tile are triangular; add -1e9 there
                                nc.tensor.matmul(
                                    sc_ps[:, off : off + 128],
                                    idnt_sb[:],
                                    negm_sb[:],
                                    start=False,
                                    stop=True,
                                )
                            p_t = pp.tile([128, 512], BF)
                            nc.scalar.activation(
                                p_t[:, off:], sc_ps[:, off:], AF.Exp, scale=EXP_SCALE
                            )
                            pts[kc] = p_t
                        j = kc - LAG
                        if 0 <= j < kmax:
                            joff = offs[j]
                            nc.tensor.matmul(
                                den_ps[:, joff:],
                                onk_sb[:],
                                pts[j][:, joff:],
                                start=(j == 0),
                                stop=(j == kmax - 1),
                            )
                            nc.tensor.matmul(
                                ctx_ps[:, joff:],
                                v_sb[:, j, hd],
                                pts[j][:, joff:],
                                start=(j == 0),
                                stop=(j == kmax - 1),
                            )
                    lnd = dp.tile([128, 512], F32, tag="lnd", bufs=2)
                    nc.scalar.activation(lnd[:], den_ps[:], AF.Ln)
                    recb = dp.tile([128, 512], F32, tag="recb", bufs=2)
                    nc.scalar.activation(recb[:], lnd[:], AF.Exp, scale=-1.0)
                    nc.vector.tensor_tensor(
                        ctxN_sb[:, h, qs], ctx_ps[:], recb[:], ALU.mult
                    )
            outproj_group(range(16), interleaved=False)
    _split_excess_waits(nc)
    return nc


def _negm_np():
    kk = np.arange(128)[:, None]
    qq = np.arange(128)[None, :]
    return np.where(kk > qq, -1.0e9, 0.0).astype(bf16)


def kernel(**inputs):
    global LAST_RESULTS, _NC_CACHE
    x = np.asarray(inputs["x"], np.float32)
    Wq = np.asarray(inputs["Wq"], np.float32)
    bq = np.asarray(inputs["bq"], np.float32)
    Wk = np.asarray(inputs["Wk"], np.float32)
    bk = np.asarray(inputs["bk"], np.float32)
    Wv = np.asarray(inputs["Wv"], np.float32)
    bv = np.asarray(inputs["bv"], np.float32)
    Wo = np.asarray(inputs["Wo"], np.float32)
    bo = np.asarray(inputs["bo"], np.float32)

    xT = np.ascontiguousarray(x.reshape(BS, E).T)
    x8 = xT.astype(f8e4)
    xb = xT.astype(bf16)
    negm = _negm_np()
    negf = np.full((128, 128), -1.0e9, dtype=bf16)
    ident = np.eye(128, dtype=bf16)
    onesk = np.ones((128, 128), bf16)

    in_maps = []
    for c in range(NCORES):
        dsl = slice(c * DC, (c + 1) * DC)
        in_maps.append(
            {
                "x8": x8,
                "xb": xb,
                "wq8": np.ascontiguousarray(Wq[dsl].T * AQ).astype(f8e4),
                "wk8": np.ascontiguousarray(Wk[dsl].T * AK).astype(f8e4),
                "wvT": np.ascontiguousarray(Wv[dsl].T).astype(bf16),
                "woT": np.ascontiguousarray(Wo[:, dsl].T).astype(bf16),
                "bq": np.ascontiguousarray((bq[dsl] * AQ).reshape(DC, 1)),
                "bk": np.ascontiguousarray((bk[dsl] * AK).reshape(DC, 1)),
                "negm": negm,
                "negf": negf,
                "ident": ident,
                "onesk": onesk,
            }
        )

    if _NC_CACHE is None:
        _NC_CACHE = _build()
    res = run_bass_kernel_spmd(_NC_CACHE, in_maps, core_ids=list(range(NCORES)), trace=TRACE)
    LAST_RESULTS = res

    acc = None
    for r in res.results:
        yc = np.asarray(r["y"], np.float32)
        acc = yc if acc is None else acc + yc
    bo_eff = bo + bv @ Wo.T
    acc += bo_eff[None, :]
    return acc.reshape(B, S, E).astype(np.float32)
